# revision 1
# baseline (speedup 1.0000x reference)
"""Trainium2 Bass kernel for an XNOR-Net BasicBlock (dense_cnn).

Computes, for x [64,256,56,56] (NCHW):
    h = xnor_conv3x3(x, w1) -> bn1 -> hardtanh -> xnor_conv3x3 -> bn2
    out = relu(h + x)

where xnor_conv binarizes activations with sign() and weights with
sign()*mean(|w|) (per output channel).

Strategy:
  - Data-parallel over batch: 8 images per NeuronCore x 8 cores.
  - Binarized activations (+-1) are exact in bf16; conv = 9 shifted
    matmuls per 3x3 offset, accumulated in fp32 PSUM (exact integers).
  - All per-channel affine work (alpha * bn scale/bias) is folded on the
    host into per-channel (scale, bias) pairs and fused into the PSUM
    evacuation on the Scalar engine (activation func with per-partition
    scale/bias), including the re-binarization for conv2.
  - hardtanh is a no-op for the final output because conv2 only consumes
    sign(h), and clip preserves sign.

Layouts (per core):
  x DRAM     [8, 2, 128, 3136]  (img, c_blk, c_in_blk, h*w) fp32
  w DRAM     [2, 128, 9, 2, 128] (ci_blk, ci, tap, co_blk, co) bf16 sign
  cn DRAM    [2, 128, 4]         (co_blk, co, {a1,c1,a2,c2}) fp32
  out DRAM   [8, 2, 128, 3136]   (img, co_blk, co, h*w) fp32

SBUF per image: padded (58x58) bf16 sign planes for conv input and the
intermediate, ping-ponged across images. Matmul rhs uses shifted 3D APs
into the padded plane; out chunks are 8 rows x 56 cols = 448 <= 512 fp32
(one PSUM bank).
"""

import os
import numpy as np

N, C, H, W = 64, 256, 56, 56
EPS = 1e-5
N_CORES = 8
IMG_PER_CORE = N // N_CORES
A = 2                     # channel blocks of 128
HP, WP = H + 2, W + 2     # padded plane
RCH = 8                   # output rows per PSUM chunk
NCH = H // RCH            # chunks per image
CHUNK = RCH * W           # 448 fp32 <= 512 (one PSUM bank)
HW = H * W

_CACHE = {}
LAST_RESULT = None


def _build_program(n_img):
    import concourse.bacc as bacc
    import concourse.mybir as mybir
    import concourse.tile as tile

    dt = mybir.dt
    AF = mybir.ActivationFunctionType
    OP = mybir.AluOpType

    nc = bacc.Bacc("TRN2", target_bir_lowering=False, debug=False)

    x_d = nc.dram_tensor("x", [n_img, A, 128, HW], dt.float32, kind="ExternalInput")
    w1_d = nc.dram_tensor("w1t", [A, 128, 9, A, 128], dt.bfloat16, kind="ExternalInput")
    w2_d = nc.dram_tensor("w2t", [A, 128, 9, A, 128], dt.bfloat16, kind="ExternalInput")
    cn_d = nc.dram_tensor("cn", [A, 128, 4], dt.float32, kind="ExternalInput")
    out_d = nc.dram_tensor("out", [n_img, A, 128, HW], dt.float32, kind="ExternalOutput")

    with tile.TileContext(nc) as tc:
        with (
            tc.tile_pool(name="consts", bufs=1) as consts,
            tc.tile_pool(name="planes", bufs=1) as planes,
            tc.tile_pool(name="xin", bufs=2) as xin,
            tc.tile_pool(name="outp", bufs=1) as outp,
            tc.tile_pool(name="evac", bufs=3) as evac,
            tc.tile_pool(name="psum", bufs=4, space="PSUM") as psum,
        ):
            ws = {}
            for conv, w_d in ((0, w1_d), (1, w2_d)):
                for a in range(A):
                    t = consts.tile([128, 9, A, 128], dt.bfloat16, tag=f"w{conv}_{a}",
                                    name=f"w{conv}_{a}")
                    nc.gpsimd.dma_start(out=t[:], in_=w_d[a])
                    ws[(conv, a)] = t
            cns = []
            for b in range(A):
                t = consts.tile([128, 4], dt.float32, tag=f"cn_{b}", name=f"cn_{b}")
                nc.gpsimd.dma_start(out=t[:], in_=cn_d[b])
                cns.append(t)

            # padded +-1 planes (borders stay 0 = conv zero-padding),
            # ping-ponged across images
            bxp = [planes.tile([128, A, HP, WP], dt.bfloat16, tag=f"bxp{j}",
                               name=f"bxp{j}") for j in range(2)]
            s2p = [planes.tile([128, A, HP, WP], dt.bfloat16, tag=f"s2p{j}",
                               name=f"s2p{j}") for j in range(2)]
            for t in (*bxp, *s2p):
                nc.vector.memset(t[:], 0.0)

            def conv_mms(ps, src_plane, conv, b, r0):
                k = 0
                for t_ in range(9):
                    ki, kj = divmod(t_, 3)
                    for a in range(A):
                        nc.tensor.matmul(
                            ps[:],
                            lhsT=ws[(conv, a)][:, t_, b, :],
                            rhs=src_plane[:, a, r0 + ki:r0 + ki + RCH, kj:kj + W],
                            start=(k == 0),
                            stop=(k == 17),
                        )
                        k += 1

            for i in range(n_img):
                j = i % 2
                x_t = xin.tile([128, A, HW], dt.float32, tag="x_t", name=f"x_{i}")
                nc.gpsimd.dma_start(out=x_t[:], in_=x_d[i].rearrange("a k s -> k a s"))

                # binarize input
                for a in range(A):
                    nc.scalar.activation(
                        out=bxp[j][:, a, 1:1 + H, 1:1 + W],
                        in_=x_t[:, a, :].rearrange("p (r c) -> p r c", c=W),
                        func=AF.Sign,
                    )

                # conv1 -> fused bn1+sign -> s2p
                for b in range(A):
                    for ch in range(NCH):
                        r0 = ch * RCH
                        ps = psum.tile([128, CHUNK], dt.float32, tag="ps",
                                       name=f"ps1_{i}_{b}_{ch}")
                        conv_mms(ps, bxp[j], 0, b, r0)
                        nc.scalar.activation(
                            out=s2p[j][:, b, 1 + r0:1 + r0 + RCH, 1:1 + W],
                            in_=ps.rearrange("p (r c) -> p r c", c=W),
                            func=AF.Sign,
                            bias=cns[b][:, 1:2],
                            scale=cns[b][:, 0:1],
                        )

                out_t = outp.tile([128, A, HW], dt.float32, tag="out_t", name=f"out_{i}")

                # conv2 -> fused bn2 + residual + relu
                for b in range(A):
                    for ch in range(NCH):
                        r0 = ch * RCH
                        ps = psum.tile([128, CHUNK], dt.float32, tag="ps",
                                       name=f"ps2_{i}_{b}_{ch}")
                        conv_mms(ps, s2p[j], 1, b, r0)
                        tt = evac.tile([128, CHUNK], dt.float32, tag="tt",
                                       name=f"tt_{i}_{b}_{ch}")
                        # tt = a2 * conv2 + c2
                        nc.scalar.activation(
                            out=tt[:],
                            in_=ps[:],
                            func=AF.Identity,
                            bias=cns[b][:, 3:4],
                            scale=cns[b][:, 2:3],
                        )
                        rr = evac.tile([128, CHUNK], dt.float32, tag="rr",
                                       name=f"rr_{i}_{b}_{ch}")
                        nc.vector.tensor_add(rr[:], tt[:], x_t[:, b, r0 * W:(r0 + RCH) * W])
                        nc.vector.tensor_scalar_max(
                            out_t[:, b, r0 * W:(r0 + RCH) * W], rr[:], 0.0)

                nc.gpsimd.dma_start(out=out_d[i].rearrange("a k s -> k a s"), in_=out_t[:])

    nc.compile()
    return nc


def _get_program(n_img):
    if n_img not in _CACHE:
        _CACHE[n_img] = _build_program(n_img)
    return _CACHE[n_img]


def _prep_consts(w1, gamma1, beta1, mean1, var1, w2, gamma2, beta2, mean2, var2):
    import ml_dtypes

    def wprep(w):
        # [O, C, 3, 3] -> [ci_blk a, ci k, tap, co_blk b, co m], sign in bf16
        s = np.sign(w.astype(np.float32)).reshape(A, 128, A, 128, 9)  # [b, m, a, k, t]
        return np.ascontiguousarray(s.transpose(2, 3, 4, 0, 1)).astype(ml_dtypes.bfloat16)

    def bnfold(w, gamma, beta, mean, var):
        alpha = np.mean(np.abs(w.astype(np.float32)), axis=(1, 2, 3), dtype=np.float32)
        inv = (gamma.astype(np.float32)
               * (1.0 / np.sqrt(var.astype(np.float64) + EPS)).astype(np.float32))
        scale = alpha * inv
        bias = beta.astype(np.float32) - mean.astype(np.float32) * inv
        return scale, bias

    a1, c1 = bnfold(w1, gamma1, beta1, mean1, var1)
    a2, c2 = bnfold(w2, gamma2, beta2, mean2, var2)
    cn = np.ascontiguousarray(
        np.stack([a1, c1, a2, c2], axis=1).reshape(A, 128, 4)).astype(np.float32)
    return wprep(w1), wprep(w2), cn


def kernel(x, w1, gamma1, beta1, mean1, var1, w2, gamma2, beta2, mean2, var2):
    global LAST_RESULT
    from concourse.bass_utils import run_bass_kernel_spmd

    nc = _get_program(IMG_PER_CORE)
    w1t, w2t, cn = _prep_consts(w1, gamma1, beta1, mean1, var1,
                                w2, gamma2, beta2, mean2, var2)

    x = np.asarray(x, dtype=np.float32)
    xs = x.reshape(N_CORES, IMG_PER_CORE, A, 128, HW)
    in_maps = [
        {"x": xs[g], "w1t": w1t, "w2t": w2t, "cn": cn} for g in range(N_CORES)
    ]

    kwargs = {}
    if os.environ.get("BASS_KERNEL_TRACE"):
        _install_trace_shim()
        kwargs = dict(trace=True, tmpdir=os.environ.get("BASS_KERNEL_TRACE_DIR") or None)

    res = run_bass_kernel_spmd(nc, in_maps, list(range(N_CORES)), **kwargs)
    LAST_RESULT = res

    out = np.empty((N, C, H, W), dtype=np.float32)
    for g in range(N_CORES):
        out[g * IMG_PER_CORE:(g + 1) * IMG_PER_CORE] = (
            res.results[g]["out"].reshape(IMG_PER_CORE, C, H, W))
    return out


def _install_trace_shim():
    """This image lacks antenv.axon_hooks; recreate it so NTFF tracing works."""
    import sys, types
    if "antenv.axon_hooks" in sys.modules:
        return
    try:
        import antenv
        from trn_agent_boot.trn_boot import _ntff_profile_via_ctypes
    except ImportError:
        return
    mod = types.ModuleType("antenv.axon_hooks")
    _hook = [_ntff_profile_via_ctypes("/opt/axon/libaxon_pjrt.so")]
    mod.set_axon_ntff_profile_hook = lambda h: _hook.__setitem__(0, h)
    mod.get_axon_ntff_profile_hook = lambda: _hook[0]
    sys.modules["antenv.axon_hooks"] = mod
    antenv.axon_hooks = mod


# revision 2
# speedup vs baseline: 1.1470x; 1.1470x over previous
"""Trainium2 Bass kernel for an XNOR-Net BasicBlock (dense_cnn).

Computes, for x [64,256,56,56] (NCHW):
    h = xnor_conv3x3(x, w1) -> bn1 -> hardtanh -> xnor_conv3x3 -> bn2
    out = relu(h + x)

where xnor_conv binarizes activations with sign() and weights with
sign()*mean(|w|) (per output channel).

Strategy (v2, fp8 DoubleRow):
  - Data-parallel over batch: 8 images per NeuronCore x 8 cores.
  - Binarized activations (+-1) are exact in fp8e4; conv = 9 shifted
    matmuls per 3x3 tap with fp32 PSUM accumulation (exact integers).
  - perf_mode=DoubleRow contracts K=256 (both 128-channel blocks) per
    matmul: lhsT [128,2,128], rhs [128,2,448]. DoubleRow requires a 3D
    rhs AP with contiguous N, so sign planes are stored 3x, one copy per
    kj column shift, with row stride 56 (58 rows x 56 cols, borders 0).
    Window for tap (ki,kj), out-row-chunk r0 is then the contiguous run
    plane[kj][:, :, (r0+ki)*56 : +448].
  - Per-channel affine work (alpha * bn scale/bias) is folded on the host
    into (scale, bias) pairs and fused into the PSUM evacuation on the
    Scalar engine, including the re-binarization for conv2. hardtanh is
    a no-op for the final output because conv2 only consumes sign(h).
  - Engine split: ACT = binarize + bn evacuations (kj=1 plane); GpSimd =
    whole-plane shifted copies for conv1 input; DVE = per-chunk shifted
    copies for conv2 input + residual add + relu.

Layouts (per core):
  x DRAM     [8, 2, 128, 3136]   (img, c_blk, c_in_blk, h*w) fp32
  w DRAM     [2, 128, 9, 2, 128] (co_blk, ci, tap, ci_blk, co) fp8 sign
  cn DRAM    [2, 128, 4]         (co_blk, co, {a1,c1,a2,c2}) fp32
  out DRAM   [8, 2, 128, 3136]   (img, co_blk, co, h*w) fp32
"""

import os
import numpy as np

N, C, H, W = 64, 256, 56, 56
EPS = 1e-5
N_CORES = 8
IMG_PER_CORE = N // N_CORES
A = 2                     # channel blocks of 128
ROWS = H + 2              # padded rows in a plane
RCH = 8                   # output rows per PSUM chunk
NCH = H // RCH            # chunks per image
CHUNK = RCH * W           # 448 fp32 <= 512 (one PSUM bank)
HW = H * W
GROUPS = [(0, 1), (2, 3), (4, 5), (6,)]   # chunk pairs sharing LDWEIGHTS

_CACHE = {}
LAST_RESULT = None


def _build_program(n_img):
    import concourse.bacc as bacc
    import concourse.mybir as mybir
    import concourse.tile as tile

    dt = mybir.dt
    AF = mybir.ActivationFunctionType
    DR = mybir.MatmulPerfMode.DoubleRow

    nc = bacc.Bacc("TRN2", target_bir_lowering=False, debug=False)

    x_d = nc.dram_tensor("x", [n_img, A, 128, HW], dt.float32, kind="ExternalInput")
    w1_d = nc.dram_tensor("w1t", [A, 128, 9, A, 128], dt.float8e4, kind="ExternalInput")
    w2_d = nc.dram_tensor("w2t", [A, 128, 9, A, 128], dt.float8e4, kind="ExternalInput")
    cn_d = nc.dram_tensor("cn", [A, 128, 4], dt.float32, kind="ExternalInput")
    out_d = nc.dram_tensor("out", [n_img, A, 128, HW], dt.float32, kind="ExternalOutput")

    with tile.TileContext(nc) as tc:
        with (
            tc.tile_pool(name="consts", bufs=1) as consts,
            tc.tile_pool(name="planes", bufs=1) as planes,
            tc.tile_pool(name="xin", bufs=2) as xin,
            tc.tile_pool(name="outp", bufs=1) as outp,
            tc.tile_pool(name="evac", bufs=3) as evac,
            tc.tile_pool(name="psum", bufs=4, space="PSUM") as psum,
        ):
            ws = {}
            for conv, w_d in ((0, w1_d), (1, w2_d)):
                for b in range(A):
                    t = consts.tile([128, 9, A, 128], dt.float8e4, tag=f"w{conv}_{b}",
                                    name=f"w{conv}_{b}")
                    nc.gpsimd.dma_start(out=t[:], in_=w_d[b])
                    ws[(conv, b)] = t
            cns = []
            for b in range(A):
                t = consts.tile([128, 4], dt.float32, tag=f"cn_{b}", name=f"cn_{b}")
                nc.gpsimd.dma_start(out=t[:], in_=cn_d[b])
                cns.append(t)

            # sign planes [128, kj, c_blk, 58 rows, 56 cols] fp8, borders 0,
            # ping-ponged across images. plane[kj][.., rr, j] = xpad[.., rr, j+kj]
            bxp = [planes.tile([128, 3, A, ROWS, W], dt.float8e4, tag=f"bxp{j}",
                               name=f"bxp{j}") for j in range(2)]
            s2p = [planes.tile([128, 3, A, ROWS, W], dt.float8e4, tag=f"s2p{j}",
                               name=f"s2p{j}") for j in range(2)]
            for t in (*bxp, *s2p):
                nc.vector.memset(t[:], 0.0)

            def conv_group(src, conv, b, group, pss):
                # rhs: contiguous-N window per tap; one DoubleRow MM contracts K=256
                flat = src.rearrange("p kj a r c -> p kj a (r c)")
                for t_ in range(9):
                    ki, kj = divmod(t_, 3)
                    for gi, ch in enumerate(group):
                        r0 = ch * RCH
                        nc.tensor.matmul(
                            pss[gi][:],
                            lhsT=ws[(conv, b)][:, t_, :, :],
                            rhs=flat[:, kj, :, (r0 + ki) * W:(r0 + ki) * W + CHUNK],
                            start=(t_ == 0), stop=(t_ == 8),
                            perf_mode=DR,
                        )

            for i in range(n_img):
                j = i % 2
                x_t = xin.tile([128, A, HW], dt.float32, tag="x_t", name=f"x_{i}")
                nc.gpsimd.dma_start(out=x_t[:], in_=x_d[i].rearrange("a k s -> k a s"))

                # binarize input into kj=1 plane (no column shift)
                for a in range(A):
                    nc.scalar.activation(
                        out=bxp[j][:, 1, a, 1:1 + H, :],
                        in_=x_t[:, a, :].rearrange("p (r c) -> p r c", c=W),
                        func=AF.Sign,
                    )
                # shifted copies for kj=0 / kj=2 (GpSimd, whole plane)
                nc.gpsimd.tensor_copy(out=bxp[j][:, 0, :, 1:1 + H, 1:W],
                                      in_=bxp[j][:, 1, :, 1:1 + H, 0:W - 1])
                nc.gpsimd.tensor_copy(out=bxp[j][:, 2, :, 1:1 + H, 0:W - 1],
                                      in_=bxp[j][:, 1, :, 1:1 + H, 1:W])

                # conv1 -> fused bn1+sign -> s2p (x3 shifted)
                for b in range(A):
                    for group in GROUPS:
                        pss = [psum.tile([128, CHUNK], dt.float32, tag="ps",
                                         name=f"ps1_{i}_{b}_{ch}") for ch in group]
                        conv_group(bxp[j], 0, b, group, pss)
                        for gi, ch in enumerate(group):
                            r0 = ch * RCH
                            nc.scalar.activation(
                                out=s2p[j][:, 1, b, 1 + r0:1 + r0 + RCH, :],
                                in_=pss[gi].rearrange("p (r c) -> p r c", c=W),
                                func=AF.Sign,
                                bias=cns[b][:, 1:2],
                                scale=cns[b][:, 0:1],
                            )
                            nc.vector.tensor_copy(
                                out=s2p[j][:, 0, b, 1 + r0:1 + r0 + RCH, 1:W],
                                in_=s2p[j][:, 1, b, 1 + r0:1 + r0 + RCH, 0:W - 1])
                            nc.vector.tensor_copy(
                                out=s2p[j][:, 2, b, 1 + r0:1 + r0 + RCH, 0:W - 1],
                                in_=s2p[j][:, 1, b, 1 + r0:1 + r0 + RCH, 1:W])

                out_t = outp.tile([128, A, HW], dt.float32, tag="out_t", name=f"out_{i}")

                # conv2 -> fused bn2 + residual + relu
                for b in range(A):
                    for group in GROUPS:
                        pss = [psum.tile([128, CHUNK], dt.float32, tag="ps",
                                         name=f"ps2_{i}_{b}_{ch}") for ch in group]
                        conv_group(s2p[j], 1, b, group, pss)
                        for gi, ch in enumerate(group):
                            r0 = ch * RCH
                            tt = evac.tile([128, CHUNK], dt.float32, tag="tt",
                                           name=f"tt_{i}_{b}_{ch}")
                            nc.scalar.activation(
                                out=tt[:],
                                in_=pss[gi][:],
                                func=AF.Identity,
                                bias=cns[b][:, 3:4],
                                scale=cns[b][:, 2:3],
                            )
                            rr = evac.tile([128, CHUNK], dt.float32, tag="rr",
                                           name=f"rr_{i}_{b}_{ch}")
                            nc.vector.tensor_add(
                                rr[:], tt[:], x_t[:, b, r0 * W:(r0 + RCH) * W])
                            nc.vector.tensor_scalar_max(
                                out_t[:, b, r0 * W:(r0 + RCH) * W], rr[:], 0.0)

                nc.gpsimd.dma_start(out=out_d[i].rearrange("a k s -> k a s"), in_=out_t[:])

    nc.compile()
    return nc


def _get_program(n_img):
    if n_img not in _CACHE:
        _CACHE[n_img] = _build_program(n_img)
    return _CACHE[n_img]


def _prep_consts(w1, gamma1, beta1, mean1, var1, w2, gamma2, beta2, mean2, var2):
    import ml_dtypes

    def wprep(w):
        # [O, C, 3, 3] -> [co_blk b, ci k, tap t, ci_blk i, co m], sign in fp8e4
        s = np.sign(w.astype(np.float32)).reshape(A, 128, A, 128, 9)  # [b, m, i, k, t]
        return np.ascontiguousarray(s.transpose(0, 3, 4, 2, 1)).astype(
            ml_dtypes.float8_e4m3)

    def bnfold(w, gamma, beta, mean, var):
        alpha = np.mean(np.abs(w.astype(np.float32)), axis=(1, 2, 3), dtype=np.float32)
        inv = (gamma.astype(np.float32)
               * (1.0 / np.sqrt(var.astype(np.float64) + EPS)).astype(np.float32))
        scale = alpha * inv
        bias = beta.astype(np.float32) - mean.astype(np.float32) * inv
        return scale, bias

    a1, c1 = bnfold(w1, gamma1, beta1, mean1, var1)
    a2, c2 = bnfold(w2, gamma2, beta2, mean2, var2)
    cn = np.ascontiguousarray(
        np.stack([a1, c1, a2, c2], axis=1).reshape(A, 128, 4)).astype(np.float32)
    return wprep(w1), wprep(w2), cn


def kernel(x, w1, gamma1, beta1, mean1, var1, w2, gamma2, beta2, mean2, var2):
    global LAST_RESULT
    from concourse.bass_utils import run_bass_kernel_spmd

    nc = _get_program(IMG_PER_CORE)
    w1t, w2t, cn = _prep_consts(w1, gamma1, beta1, mean1, var1,
                                w2, gamma2, beta2, mean2, var2)

    x = np.asarray(x, dtype=np.float32)
    xs = x.reshape(N_CORES, IMG_PER_CORE, A, 128, HW)
    in_maps = [
        {"x": xs[g], "w1t": w1t, "w2t": w2t, "cn": cn} for g in range(N_CORES)
    ]

    kwargs = {}
    if os.environ.get("BASS_KERNEL_TRACE"):
        _install_trace_shim()
        kwargs = dict(trace=True, tmpdir=os.environ.get("BASS_KERNEL_TRACE_DIR") or None)

    res = run_bass_kernel_spmd(nc, in_maps, list(range(N_CORES)), **kwargs)
    LAST_RESULT = res

    out = np.empty((N, C, H, W), dtype=np.float32)
    for g in range(N_CORES):
        out[g * IMG_PER_CORE:(g + 1) * IMG_PER_CORE] = (
            res.results[g]["out"].reshape(IMG_PER_CORE, C, H, W))
    return out


def _install_trace_shim():
    """This image lacks antenv.axon_hooks; recreate it so NTFF tracing works."""
    import sys, types
    if "antenv.axon_hooks" in sys.modules:
        return
    try:
        import antenv
        from trn_agent_boot.trn_boot import _ntff_profile_via_ctypes
    except ImportError:
        return
    mod = types.ModuleType("antenv.axon_hooks")
    _hook = [_ntff_profile_via_ctypes("/opt/axon/libaxon_pjrt.so")]
    mod.set_axon_ntff_profile_hook = lambda h: _hook.__setitem__(0, h)
    mod.get_axon_ntff_profile_hook = lambda: _hook[0]
    sys.modules["antenv.axon_hooks"] = mod
    antenv.axon_hooks = mod


# revision 3
# speedup vs baseline: 1.2201x; 1.0637x over previous
"""Trainium2 Bass kernel for an XNOR-Net BasicBlock (dense_cnn).

Computes, for x [64,256,56,56] (NCHW):
    h = xnor_conv3x3(x, w1) -> bn1 -> hardtanh -> xnor_conv3x3 -> bn2
    out = relu(h + x)

where xnor_conv binarizes activations with sign() and weights with
sign()*mean(|w|) (per output channel).

Strategy (v3, fp8 DoubleRow):
  - Data-parallel over batch: 8 images per NeuronCore x 8 cores.
  - Binarized activations (+-1) are exact in fp8e4; conv = 9 shifted
    matmuls per 3x3 tap with fp32 PSUM accumulation (exact integers).
  - perf_mode=DoubleRow contracts K=256 (both 128-channel blocks) per
    matmul: lhsT [128,2,128], rhs [128,2,448]. DoubleRow requires a 3D
    rhs AP with contiguous N, so sign planes are stored 3x, one copy per
    kj column shift, with row stride 56 (58 rows x 56 cols, borders 0).
    Window for tap (ki,kj), out-row-chunk r0 is then the contiguous run
    plane[kj][:, :, (r0+ki)*W : +448].
  - The kj=0/2 copies of the conv1 input planes are made by GpSimd as
    contiguous flat shifts by +-1 element (the shifted-in column of
    garbage is re-zeroed with a tiny strided memset; it corresponds to
    the zero padding column). conv2 input copies are small per-chunk
    strided DVE copies.
  - Residual: a diag(1/a2) fp32 matmul accumulates x/a2 into the conv2
    PSUM group, so the whole conv2 epilogue (bn2 + residual + relu) is a
    single fused ScalarE op: relu(a2*psum + c2). All per-channel affine
    constants (alpha, bn scale/bias) are folded on the host. hardtanh is
    a no-op for the final output because conv2 only consumes sign(h).

Layouts (per core):
  x DRAM     [8, 2, 128, 3136]   (img, c_blk, c_in_blk, h*w) fp32
  w DRAM     [2, 128, 9, 2, 128] (co_blk, ci, tap, ci_blk, co) fp8 sign
  cn DRAM    [2, 128, 4]         (co_blk, co, {a1,c1,a2,c2}) fp32
  dg DRAM    [2, 128, 128]       diag(1/a2) per co_blk, fp32
  out DRAM   [8, 2, 128, 3136]   (img, co_blk, co, h*w) fp32
"""

import os
import numpy as np

N, C, H, W = 64, 256, 56, 56
EPS = 1e-5
N_CORES = 8
IMG_PER_CORE = N // N_CORES
A = 2                     # channel blocks of 128
ROWS = H + 2              # padded rows in a plane
PLANE = ROWS * W          # 3248 (multiple of 16 for DoubleRow dim1 step)
RCH = 8                   # output rows per PSUM chunk
NCH = H // RCH            # chunks per image
CHUNK = RCH * W           # 448 fp32 <= 512 (one PSUM bank)
HW = H * W
GROUPS = [(0, 1), (2, 3), (4, 5), (6,)]   # chunk pairs sharing LDWEIGHTS

_CACHE = {}
LAST_RESULT = None


def _build_program(n_img):
    import concourse.bacc as bacc
    import concourse.mybir as mybir
    import concourse.tile as tile

    dt = mybir.dt
    AF = mybir.ActivationFunctionType
    DR = mybir.MatmulPerfMode.DoubleRow

    nc = bacc.Bacc("TRN2", target_bir_lowering=False, debug=False)

    x_d = nc.dram_tensor("x", [n_img, A, 128, HW], dt.float32, kind="ExternalInput")
    w1_d = nc.dram_tensor("w1t", [A, 128, 9, A, 128], dt.float8e4, kind="ExternalInput")
    w2_d = nc.dram_tensor("w2t", [A, 128, 9, A, 128], dt.float8e4, kind="ExternalInput")
    cn_d = nc.dram_tensor("cn", [A, 128, 4], dt.float32, kind="ExternalInput")
    dg_d = nc.dram_tensor("dg", [A, 128, 128], dt.float32, kind="ExternalInput")
    out_d = nc.dram_tensor("out", [n_img, A, 128, HW], dt.float32, kind="ExternalOutput")

    with tile.TileContext(nc) as tc:
        with (
            tc.tile_pool(name="consts", bufs=1) as consts,
            tc.tile_pool(name="planes", bufs=1) as planes,
            tc.tile_pool(name="xin", bufs=2) as xin,
            tc.tile_pool(name="outp", bufs=1) as outp,
            tc.tile_pool(name="psum", bufs=6, space="PSUM") as psum,
        ):
            ws = {}
            for conv, w_d in ((0, w1_d), (1, w2_d)):
                for b in range(A):
                    t = consts.tile([128, 9, A, 128], dt.float8e4, tag=f"w{conv}_{b}",
                                    name=f"w{conv}_{b}")
                    nc.gpsimd.dma_start(out=t[:], in_=w_d[b])
                    ws[(conv, b)] = t
            cns = []
            dgs = []
            for b in range(A):
                t = consts.tile([128, 4], dt.float32, tag=f"cn_{b}", name=f"cn_{b}")
                nc.gpsimd.dma_start(out=t[:], in_=cn_d[b])
                cns.append(t)
                t = consts.tile([128, 128], dt.float32, tag=f"dg_{b}", name=f"dg_{b}")
                nc.gpsimd.dma_start(out=t[:], in_=dg_d[b])
                dgs.append(t)

            # sign planes [128, kj, c_blk, 58 rows, 56 cols] fp8, borders 0,
            # ping-ponged across images. plane[kj][.., rr, j] = xpad[.., rr, j+kj]
            bxp = [planes.tile([128, 3, A, ROWS, W], dt.float8e4, tag=f"bxp{j}",
                               name=f"bxp{j}") for j in range(2)]
            s2p = [planes.tile([128, 3, A, ROWS, W], dt.float8e4, tag=f"s2p{j}",
                               name=f"s2p{j}") for j in range(2)]
            for t in (*bxp, *s2p):
                # border-only init: zero rows 0/57 (all kj) and the padding
                # columns never overwritten per image (kj0 col 0, kj2 col W-1)
                nc.vector.memset(t[:, :, :, 0, :], 0.0)
                nc.vector.memset(t[:, :, :, ROWS - 1, :], 0.0)
                nc.vector.memset(t[:, 0, :, :, 0:1], 0.0)
                nc.vector.memset(t[:, 2, :, :, W - 1:W], 0.0)

            def conv_group(src, conv, b, group, pss, x_t):
                flat = src.rearrange("p kj a r c -> p kj a (r c)")
                for gi, ch in enumerate(group):
                    # residual (conv2 only): psum = x/a2, taps accumulate on top
                    if x_t is not None:
                        r0 = ch * RCH
                        nc.tensor.matmul(
                            pss[gi][:], lhsT=dgs[b][:],
                            rhs=x_t[:, b, r0 * W:(r0 + RCH) * W],
                            start=True, stop=False)
                for t_ in range(9):
                    ki, kj = divmod(t_, 3)
                    for gi, ch in enumerate(group):
                        r0 = ch * RCH
                        nc.tensor.matmul(
                            pss[gi][:],
                            lhsT=ws[(conv, b)][:, t_, :, :],
                            rhs=flat[:, kj, :, (r0 + ki) * W:(r0 + ki) * W + CHUNK],
                            start=(t_ == 0 and x_t is None), stop=(t_ == 8),
                            perf_mode=DR,
                        )

            for i in range(n_img):
                j = i % 2
                x_t = xin.tile([128, A, HW], dt.float32, tag="x_t", name=f"x_{i}")
                nc.gpsimd.dma_start(out=x_t[:], in_=x_d[i].rearrange("a k s -> k a s"))

                # binarize input into kj=1 plane (no column shift)
                for a in range(A):
                    nc.scalar.activation(
                        out=bxp[j][:, 1, a, 1:1 + H, :],
                        in_=x_t[:, a, :].rearrange("p (r c) -> p r c", c=W),
                        func=AF.Sign,
                    )
                # kj=0 / kj=2 planes: contiguous flat shift by +-1 element,
                # then re-zero the wrapped-in padding column
                bxf = bxp[j].rearrange("p kj a r c -> p kj a (r c)")
                nc.gpsimd.tensor_copy(out=bxf[:, 0, :, 1:PLANE],
                                      in_=bxf[:, 1, :, 0:PLANE - 1])
                nc.gpsimd.memset(bxp[j][:, 0, :, :, 0:1], 0.0)
                nc.gpsimd.tensor_copy(out=bxf[:, 2, :, 0:PLANE - 1],
                                      in_=bxf[:, 1, :, 1:PLANE])
                nc.gpsimd.memset(bxp[j][:, 2, :, :, W - 1:W], 0.0)

                # conv1 -> fused bn1+sign -> s2p (x3 shifted)
                for b in range(A):
                    for group in GROUPS:
                        pss = [psum.tile([128, CHUNK], dt.float32, tag="ps",
                                         name=f"ps1_{i}_{b}_{ch}") for ch in group]
                        conv_group(bxp[j], 0, b, group, pss, None)
                        for gi, ch in enumerate(group):
                            r0 = ch * RCH
                            nc.scalar.activation(
                                out=s2p[j][:, 1, b, 1 + r0:1 + r0 + RCH, :],
                                in_=pss[gi].rearrange("p (r c) -> p r c", c=W),
                                func=AF.Sign,
                                bias=cns[b][:, 1:2],
                                scale=cns[b][:, 0:1],
                            )
                            nc.vector.tensor_copy(
                                out=s2p[j][:, 0, b, 1 + r0:1 + r0 + RCH, 1:W],
                                in_=s2p[j][:, 1, b, 1 + r0:1 + r0 + RCH, 0:W - 1])
                            nc.vector.tensor_copy(
                                out=s2p[j][:, 2, b, 1 + r0:1 + r0 + RCH, 0:W - 1],
                                in_=s2p[j][:, 1, b, 1 + r0:1 + r0 + RCH, 1:W])

                out_t = outp.tile([128, A, HW], dt.float32, tag="out_t", name=f"out_{i}")

                # conv2 (+ residual in PSUM) -> single fused relu(a2*psum + c2)
                for b in range(A):
                    for group in GROUPS:
                        pss = [psum.tile([128, CHUNK], dt.float32, tag="ps",
                                         name=f"ps2_{i}_{b}_{ch}") for ch in group]
                        conv_group(s2p[j], 1, b, group, pss, x_t)
                        for gi, ch in enumerate(group):
                            r0 = ch * RCH
                            nc.scalar.activation(
                                out=out_t[:, b, r0 * W:(r0 + RCH) * W],
                                in_=pss[gi][:],
                                func=AF.Relu,
                                bias=cns[b][:, 3:4],
                                scale=cns[b][:, 2:3],
                            )

                nc.gpsimd.dma_start(out=out_d[i].rearrange("a k s -> k a s"), in_=out_t[:])

    nc.compile()
    return nc


def _get_program(n_img):
    if n_img not in _CACHE:
        _CACHE[n_img] = _build_program(n_img)
    return _CACHE[n_img]


def _prep_consts(w1, gamma1, beta1, mean1, var1, w2, gamma2, beta2, mean2, var2):
    import ml_dtypes

    def wprep(w):
        # [O, C, 3, 3] -> [co_blk b, ci k, tap t, ci_blk i, co m], sign in fp8e4
        s = np.sign(w.astype(np.float32)).reshape(A, 128, A, 128, 9)  # [b, m, i, k, t]
        return np.ascontiguousarray(s.transpose(0, 3, 4, 2, 1)).astype(
            ml_dtypes.float8_e4m3)

    def bnfold(w, gamma, beta, mean, var):
        alpha = np.mean(np.abs(w.astype(np.float32)), axis=(1, 2, 3), dtype=np.float32)
        inv = (gamma.astype(np.float32)
               * (1.0 / np.sqrt(var.astype(np.float64) + EPS)).astype(np.float32))
        scale = alpha * inv
        bias = beta.astype(np.float32) - mean.astype(np.float32) * inv
        return scale, bias

    a1, c1 = bnfold(w1, gamma1, beta1, mean1, var1)
    a2, c2 = bnfold(w2, gamma2, beta2, mean2, var2)
    cn = np.ascontiguousarray(
        np.stack([a1, c1, a2, c2], axis=1).reshape(A, 128, 4)).astype(np.float32)
    dg = np.zeros((A, 128, 128), np.float32)
    inv_a2 = (1.0 / a2.astype(np.float64)).astype(np.float32)
    for b in range(A):
        np.fill_diagonal(dg[b], inv_a2[b * 128:(b + 1) * 128])
    return wprep(w1), wprep(w2), cn, dg


def kernel(x, w1, gamma1, beta1, mean1, var1, w2, gamma2, beta2, mean2, var2):
    global LAST_RESULT
    from concourse.bass_utils import run_bass_kernel_spmd

    nc = _get_program(IMG_PER_CORE)
    w1t, w2t, cn, dg = _prep_consts(w1, gamma1, beta1, mean1, var1,
                                    w2, gamma2, beta2, mean2, var2)

    x = np.asarray(x, dtype=np.float32)
    xs = x.reshape(N_CORES, IMG_PER_CORE, A, 128, HW)
    in_maps = [
        {"x": xs[g], "w1t": w1t, "w2t": w2t, "cn": cn, "dg": dg}
        for g in range(N_CORES)
    ]

    kwargs = {}
    if os.environ.get("BASS_KERNEL_TRACE"):
        _install_trace_shim()
        kwargs = dict(trace=True, tmpdir=os.environ.get("BASS_KERNEL_TRACE_DIR") or None)

    res = run_bass_kernel_spmd(nc, in_maps, list(range(N_CORES)), **kwargs)
    LAST_RESULT = res

    out = np.empty((N, C, H, W), dtype=np.float32)
    for g in range(N_CORES):
        out[g * IMG_PER_CORE:(g + 1) * IMG_PER_CORE] = (
            res.results[g]["out"].reshape(IMG_PER_CORE, C, H, W))
    return out


def _install_trace_shim():
    """This image lacks antenv.axon_hooks; recreate it so NTFF tracing works."""
    import sys, types
    if "antenv.axon_hooks" in sys.modules:
        return
    try:
        import antenv
        from trn_agent_boot.trn_boot import _ntff_profile_via_ctypes
    except ImportError:
        return
    mod = types.ModuleType("antenv.axon_hooks")
    _hook = [_ntff_profile_via_ctypes("/opt/axon/libaxon_pjrt.so")]
    mod.set_axon_ntff_profile_hook = lambda h: _hook.__setitem__(0, h)
    mod.get_axon_ntff_profile_hook = lambda: _hook[0]
    sys.modules["antenv.axon_hooks"] = mod
    antenv.axon_hooks = mod


# revision 8
# speedup vs baseline: 1.9136x; 1.5684x over previous
"""Trainium2 Bass kernel for an XNOR-Net BasicBlock (dense_cnn).

Computes, for x [64,256,56,56] (NCHW):
    h = xnor_conv3x3(x, w1) -> bn1 -> hardtanh -> xnor_conv3x3 -> bn2
    out = relu(h + x)

where xnor_conv binarizes activations with sign() and weights with
sign()*mean(|w|) (per output channel).

Strategy (v4, fp8 DoubleRow):
  - Data-parallel over batch: 8 images per NeuronCore x 8 cores.
  - Binarized activations (+-1) are exact in fp8e4; conv = 9 shifted
    matmuls per 3x3 tap with fp32 PSUM accumulation (exact integers).
  - perf_mode=DoubleRow contracts K=256 (both 128-channel blocks) per
    matmul: lhsT [128,2,128], rhs [128,2,448]. DoubleRow requires a 3D
    rhs AP with contiguous N, so sign planes are stored 3x, one copy per
    kj column shift, with row stride 56 (58 rows x 56 cols, borders 0).
    Window for tap (ki,kj), out-row-chunk r0 is then the contiguous run
    plane[kj][:, :, (r0+ki)*W : +N].
  - Chunks are processed in pairs sharing one 2-bank PSUM tile [128,896]
    (each matmul still targets a single bank), halving evacuation ops.
  - Epilogue fusions: conv1 evac = Sign(a1*psum + c1) on ScalarE writing
    the kj=1 plane (DVE makes the kj=0/2 shifted copies); conv2 evac =
    DVE (psum*a2)+x then ScalarE Relu(. + c2). All per-channel constants
    (alpha, bn scale/bias) are folded on the host. hardtanh is a no-op
    for the final output because conv2 only consumes sign(h).

Layouts (per core):
  x DRAM     [8, 2, 128, 3136]   (img, c_blk, c_in_blk, h*w) fp32
  w DRAM     [2, 128, 9, 2, 128] (co_blk, ci, tap, ci_blk, co) fp8 sign
  cn DRAM    [2, 128, 4]         (co_blk, co, {a1,c1,a2,c2}) fp32
  out DRAM   [8, 2, 128, 3136]   (img, co_blk, co, h*w) fp32
"""

import os
import numpy as np

N, C, H, W = 64, 256, 56, 56
EPS = 1e-5
N_CORES = 8
IMG_PER_CORE = N // N_CORES
A = 2                     # channel blocks of 128
ROWS = H + 2              # padded rows in a plane
PLANE = ROWS * W          # 3248 (multiple of 16 for DoubleRow dim1 step)
RCH = 8                   # output rows per PSUM chunk
CHUNK = RCH * W           # 448 fp32 <= 512 (one PSUM bank)
HW = H * W
GROUPS = [(0, 1), (2, 3), (4, 5), (6,)]   # chunk pairs -> one PSUM tile
TAPS = [1, 4, 7, 0, 3, 6, 2, 5, 8]        # kj=1 taps first (plane-prep overlap)

_CACHE = {}
LAST_RESULT = None


def _build_program(n_img):
    import concourse.bacc as bacc
    import concourse.mybir as mybir
    import concourse.tile as tile

    dt = mybir.dt
    AF = mybir.ActivationFunctionType
    OP = mybir.AluOpType
    DR = mybir.MatmulPerfMode.DoubleRow

    nc = bacc.Bacc("TRN2", target_bir_lowering=False, debug=False)

    x_d = nc.dram_tensor("x", [n_img, A, 128, HW], dt.float32, kind="ExternalInput")
    w1_d = nc.dram_tensor("w1t", [A, 128, 9, A, 128], dt.float8e4, kind="ExternalInput")
    w2_d = nc.dram_tensor("w2t", [A, 128, 9, A, 128], dt.float8e4, kind="ExternalInput")
    cn_d = nc.dram_tensor("cn", [A, 128, 4], dt.float32, kind="ExternalInput")
    out_d = nc.dram_tensor("out", [n_img, A, 128, HW], dt.float32, kind="ExternalOutput")

    with tile.TileContext(nc) as tc:
        with (
            tc.tile_pool(name="consts", bufs=1) as consts,
            tc.tile_pool(name="planes", bufs=1) as planes,
            tc.tile_pool(name="xin", bufs=2) as xin,
            tc.tile_pool(name="outp", bufs=1) as outp,
            tc.tile_pool(name="evac", bufs=3) as evac,
            tc.tile_pool(name="psum", bufs=1, space="PSUM") as psum,
        ):
            ws = {}
            for conv, w_d in ((0, w1_d), (1, w2_d)):
                for b in range(A):
                    t = consts.tile([128, 9, A, 128], dt.float8e4, tag=f"w{conv}_{b}",
                                    name=f"w{conv}_{b}")
                    nc.gpsimd.dma_start(out=t[:], in_=w_d[b])
                    ws[(conv, b)] = t
            cns = []
            for b in range(A):
                t = consts.tile([128, 4], dt.float32, tag=f"cn_{b}", name=f"cn_{b}")
                nc.gpsimd.dma_start(out=t[:], in_=cn_d[b])
                cns.append(t)

            # sign planes [128, kj, c_blk, 58 rows, 56 cols] fp8, borders 0,
            # ping-ponged across images. plane[kj][.., rr, j] = xpad[.., rr, j+kj]
            bxp = [planes.tile([128, 3, A, ROWS, W], dt.float8e4, tag=f"bxp{j}",
                               name=f"bxp{j}") for j in range(2)]
            s2p = [planes.tile([128, 3, A, ROWS, W], dt.float8e4, tag=f"s2p{j}",
                               name=f"s2p{j}") for j in range(2)]
            for t in (*bxp, *s2p):
                # border-only init: zero rows 0/57 (all kj) and the padding
                # columns never overwritten per image (kj0 col 0, kj2 col W-1)
                nc.vector.memset(t[:, :, :, 0, :], 0.0)
                nc.vector.memset(t[:, :, :, ROWS - 1, :], 0.0)
                nc.vector.memset(t[:, 0, :, :, 0:1], 0.0)
                nc.vector.memset(t[:, 2, :, :, W - 1:W], 0.0)

            BANK = 512

            def conv_group(src, conv, b, group, ps):
                flat = src.rearrange("p kj a r c -> p kj a (r c)")
                for n_, t_ in enumerate(TAPS):
                    ki, kj = divmod(t_, 3)
                    for gi, ch in enumerate(group):
                        r0 = ch * RCH
                        nc.tensor.matmul(
                            ps[:, gi * BANK:gi * BANK + CHUNK],
                            lhsT=ws[(conv, b)][:, t_, :, :],
                            rhs=flat[:, kj, :, (r0 + ki) * W:(r0 + ki) * W + CHUNK],
                            start=(n_ == 0), stop=(n_ == 8),
                            perf_mode=DR,
                        )

            def psum_tile(group, nm):
                # chunks live at bank-aligned offsets; tail 64 fp32/bank unused
                return psum.tile([128, len(group) * BANK], dt.float32,
                                 tag=f"ps{len(group)}", bufs=3 if len(group) > 1 else 2,
                                 name=nm)

            def psum_chunks(ps, group):
                # [128, G, 448] view of the used part of each bank
                return ps.rearrange("p (g x) -> p g x", x=BANK)[:, :, 0:CHUNK]

            for i in range(n_img):
                j = i % 2
                x_t = xin.tile([128, A, HW], dt.float32, tag="x_t", name=f"x_{i}")
                nc.gpsimd.dma_start(out=x_t[:], in_=x_d[i].rearrange("a k s -> k a s"))

                # binarize input: kj=1 and kj=0 planes on ScalarE, kj=2 via DVE copy
                for a in range(A):
                    nc.scalar.activation(
                        out=bxp[j][:, 1, a, 1:1 + H, :],
                        in_=x_t[:, a, :].rearrange("p (r c) -> p r c", c=W),
                        func=AF.Sign,
                    )
                for a in range(A):
                    nc.scalar.activation(
                        out=bxp[j][:, 0, a, 1:1 + H, 1:W],
                        in_=x_t[:, a, :].rearrange("p (r c) -> p r c", c=W)[:, :, 0:W - 1],
                        func=AF.Sign,
                    )
                nc.vector.tensor_copy(out=bxp[j][:, 2, :, 1:1 + H, 0:W - 1],
                                      in_=bxp[j][:, 1, :, 1:1 + H, 1:W])

                # conv1 -> fused bn1+sign -> s2p (x3 shifted)
                for b in range(A):
                    for group in GROUPS:
                        gr = len(group) * RCH
                        r0 = group[0] * RCH
                        ps = psum_tile(group, f"ps1_{i}_{b}_{group[0]}")
                        conv_group(bxp[j], 0, b, group, ps)
                        nc.scalar.activation(
                            out=s2p[j][:, 1, b, 1 + r0:1 + r0 + gr, :],
                            in_=psum_chunks(ps, group).rearrange(
                                "p g (r c) -> p g r c", c=W),
                            func=AF.Sign,
                            bias=cns[b][:, 1:2],
                            scale=cns[b][:, 0:1],
                        )
                        nc.vector.tensor_copy(
                            out=s2p[j][:, 0, b, 1 + r0:1 + r0 + gr, 1:W],
                            in_=s2p[j][:, 1, b, 1 + r0:1 + r0 + gr, 0:W - 1])
                        nc.vector.tensor_copy(
                            out=s2p[j][:, 2, b, 1 + r0:1 + r0 + gr, 0:W - 1],
                            in_=s2p[j][:, 1, b, 1 + r0:1 + r0 + gr, 1:W])

                out_t = outp.tile([128, A, HW], dt.float32, tag="out_t", name=f"out_{i}")

                # conv2 -> DVE (psum*a2)+x -> ScalarE relu(. + c2)
                for b in range(A):
                    for group in GROUPS:
                        gn = len(group) * CHUNK
                        s0 = group[0] * CHUNK
                        ps = psum_tile(group, f"ps2_{i}_{b}_{group[0]}")
                        conv_group(s2p[j], 1, b, group, ps)
                        rr = evac.tile([128, 2 * CHUNK], dt.float32, tag="rr",
                                       name=f"rr_{i}_{b}_{group[0]}")
                        nc.vector.scalar_tensor_tensor(
                            out=rr[:, :gn], in0=psum_chunks(ps, group),
                            scalar=cns[b][:, 2:3],
                            in1=x_t[:, b, s0:s0 + gn],
                            op0=OP.mult, op1=OP.add)
                        nc.scalar.activation(
                            out=out_t[:, b, s0:s0 + gn],
                            in_=rr[:, :gn],
                            func=AF.Relu,
                            bias=cns[b][:, 3:4],
                        )

                nc.gpsimd.dma_start(out=out_d[i].rearrange("a k s -> k a s"), in_=out_t[:])

    nc.compile()
    return nc


def _get_program(n_img):
    if n_img not in _CACHE:
        _CACHE[n_img] = _build_program(n_img)
    return _CACHE[n_img]


def _prep_consts(w1, gamma1, beta1, mean1, var1, w2, gamma2, beta2, mean2, var2):
    import ml_dtypes

    def wprep(w):
        # [O, C, 3, 3] -> [co_blk b, ci k, tap t, ci_blk i, co m], sign in fp8e4
        s = np.sign(w.astype(np.float32)).reshape(A, 128, A, 128, 9)  # [b, m, i, k, t]
        return np.ascontiguousarray(s.transpose(0, 3, 4, 2, 1)).astype(
            ml_dtypes.float8_e4m3)

    def bnfold(w, gamma, beta, mean, var):
        alpha = np.mean(np.abs(w.astype(np.float32)), axis=(1, 2, 3), dtype=np.float32)
        inv = (gamma.astype(np.float32)
               * (1.0 / np.sqrt(var.astype(np.float64) + EPS)).astype(np.float32))
        scale = alpha * inv
        bias = beta.astype(np.float32) - mean.astype(np.float32) * inv
        return scale, bias

    a1, c1 = bnfold(w1, gamma1, beta1, mean1, var1)
    a2, c2 = bnfold(w2, gamma2, beta2, mean2, var2)
    cn = np.ascontiguousarray(
        np.stack([a1, c1, a2, c2], axis=1).reshape(A, 128, 4)).astype(np.float32)
    return wprep(w1), wprep(w2), cn


def kernel(x, w1, gamma1, beta1, mean1, var1, w2, gamma2, beta2, mean2, var2):
    global LAST_RESULT
    from concourse.bass_utils import run_bass_kernel_spmd

    x, w1, gamma1, beta1, mean1, var1, w2, gamma2, beta2, mean2, var2 = (
        np.asarray(v) for v in
        (x, w1, gamma1, beta1, mean1, var1, w2, gamma2, beta2, mean2, var2))

    nc = _get_program(IMG_PER_CORE)
    w1t, w2t, cn = _prep_consts(w1, gamma1, beta1, mean1, var1,
                                w2, gamma2, beta2, mean2, var2)

    x = np.asarray(x, dtype=np.float32)
    xs = x.reshape(N_CORES, IMG_PER_CORE, A, 128, HW)
    in_maps = [
        {"x": xs[g], "w1t": w1t, "w2t": w2t, "cn": cn} for g in range(N_CORES)
    ]

    kwargs = {}
    if os.environ.get("BASS_KERNEL_TRACE"):
        _install_trace_shim()
        kwargs = dict(trace=True, tmpdir=os.environ.get("BASS_KERNEL_TRACE_DIR") or None)

    res = run_bass_kernel_spmd(nc, in_maps, list(range(N_CORES)), **kwargs)
    LAST_RESULT = res

    out = np.empty((N, C, H, W), dtype=np.float32)
    for g in range(N_CORES):
        out[g * IMG_PER_CORE:(g + 1) * IMG_PER_CORE] = (
            res.results[g]["out"].reshape(IMG_PER_CORE, C, H, W))
    return out


def _install_trace_shim():
    """This image lacks antenv.axon_hooks; recreate it so NTFF tracing works."""
    import sys, types
    if "antenv.axon_hooks" in sys.modules:
        return
    try:
        import antenv
        from trn_agent_boot.trn_boot import _ntff_profile_via_ctypes
    except ImportError:
        return
    mod = types.ModuleType("antenv.axon_hooks")
    _hook = [_ntff_profile_via_ctypes("/opt/axon/libaxon_pjrt.so")]
    mod.set_axon_ntff_profile_hook = lambda h: _hook.__setitem__(0, h)
    mod.get_axon_ntff_profile_hook = lambda: _hook[0]
    sys.modules["antenv.axon_hooks"] = mod
    antenv.axon_hooks = mod
